# revision 72
# baseline (speedup 1.0000x reference)
"""Trainium2 Bass kernel for nn_ConeIntersection (v2).

Pure data-parallel over B (8 cores x 1024 tokens). Host pre-transposes inputs
to [N, DIM, B_local]; concat([axis-arg/2, axis+arg/2]) folded into effective
weights; mean-over-N of the gate folded into summed h1g + scaled W.

v2 engine plan (vs baseline: DVE 400us / Act 286us / PE 189us / Pool 0):
 - Two activation tables only (exp_and_others / trig_and_small), grouped per
   2-head phase via token-activation dependency chains (baseline thrashed 65
   table loads = 83us).
 - atan2 via half-angle identity atan2(|y|,|x|) = pi/4 + atan((|y|-|x|)/(|y|+|x|)),
   constant clamp (drops the Se softmax-denominator entirely), sign applied
   by bit-or; DVE `divide` (2x mode) instead of reciprocal+mult.
 - sin/cos range reduction via 2-op tensor_scalar (add,mod) at 2x instead of
   add_range_wrap microcode; sin(x)=Sin(w-pi), cos(x)=Sin(w2-pi).
 - Pool engine (idle in baseline) takes arg-relu, e*cos/e*sin products,
   Ss/min/h1g trees and atan2 tensor-tensor ops.
 - Gate sigmoid via exp table + divide: min/(1+exp(-z)).
"""
import sys
sys.path.insert(0, '/opt/trn_rl_repo')
import numpy as np
from contextlib import ExitStack

N, B, DIM, HEADS = 4, 8192, 1024, 4
HD = DIM // HEADS            # 256
NCORES = 8
BL = B // NCORES             # 1024 tokens per core
TB = 256                     # token tile (matmul free dim; >=256 keeps f32r at 1cyc/row)
NBT = BL // TB               # 4
GH = 2                       # heads per activation-table phase group
PI = 3.141592653589793
C_ABS = 1e-6                 # abs clamp for x_emb (replaces 0.001*sum_exp; see notes)
USE_MOD_WRAP = False

_CACHE = {}


def _build():
    from concourse import bacc, tile, mybir
    f32 = mybir.dt.float32
    f32r = mybir.dt.float32r
    i32 = mybir.dt.int32
    AF = mybir.ActivationFunctionType
    ALU = mybir.AluOpType

    nc = bacc.Bacc("TRN2", target_bir_lowering=False, debug=False,
                   num_devices=NCORES)

    # host pre-lays inputs as [head, partition, (n,j), token] so one 3-dim
    # DMA loads a full head tile [128, N, 2, TB]
    axisQ_d = nc.dram_tensor("axisQ", [HEADS, 128, N * 2, BL], f32,
                             kind="ExternalInput")
    argQ_d = nc.dram_tensor("argQ", [HEADS, 128, N * 2, BL], f32,
                            kind="ExternalInput")
    # all weights packed into one dram tensor: [6 W, 2 i-chunks, 128, HD],
    # all biases into another: [4, 2, 128]
    wpack_d = nc.dram_tensor("wpack", [6, 2, 128, HD], f32, kind="ExternalInput")
    bpack_d = nc.dram_tensor("bpack", [4, 2, 128], f32, kind="ExternalInput")
    WNAMES = ["waax", "waar", "wgax", "wgar", "w2a", "w2g"]
    BNAMES = ["b1a", "b1g", "b2a", "nb2g"]
    axo_d = nc.dram_tensor("axis_outT", [DIM, BL], f32, kind="ExternalOutput")
    ago_d = nc.dram_tensor("arg_outT", [DIM, BL], f32, kind="ExternalOutput")

    fl = lambda t: t[:].rearrange("p a b c -> p (a b c)")

    with tile.TileContext(nc) as tc, ExitStack() as ctx:
        wpool = ctx.enter_context(tc.tile_pool(name="w", bufs=1))
        atp = ctx.enter_context(tc.tile_pool(name="atp", bufs=3))     # 8KB each
        gtp = ctx.enter_context(tc.tile_pool(name="gtp", bufs=2))     # 8KB
        h1p = ctx.enter_context(tc.tile_pool(name="h1p", bufs=3))     # 4KB
        expp = ctx.enter_context(tc.tile_pool(name="expp", bufs=2))   # 8KB
        wvp = ctx.enter_context(tc.tile_pool(name="wvp", bufs=2))     # 8KB wrap args
        trp = ctx.enter_context(tc.tile_pool(name="trp", bufs=3))     # 8KB trig transients
        l1p = ctx.enter_context(tc.tile_pool(name="l1p", bufs=2))     # 4KB tree lvl1
        hsp = ctx.enter_context(tc.tile_pool(name="hsp", bufs=2))     # h1g sums
        scp = ctx.enter_context(tc.tile_pool(name="scp", bufs=2))     # 4KB Sc/Ss group tiles
        bat = ctx.enter_context(tc.tile_pool(name="bat", bufs=1))     # 4KB minv/eg group
        a2p = ctx.enter_context(tc.tile_pool(name="a2p", bufs=4))     # 4KB atan2 temps
        a2i = ctx.enter_context(tc.tile_pool(name="a2i", bufs=2))     # 4KB i32 masks
        up = ctx.enter_context(tc.tile_pool(name="up", bufs=2))       # 4KB deferred u
        outp = ctx.enter_context(tc.tile_pool(name="outp", bufs=2))   # 4KB outputs
        tokp = ctx.enter_context(tc.tile_pool(name="tokp", bufs=4))   # tiny
        pmm = ctx.enter_context(tc.tile_pool(name="pmm", bufs=2, space="PSUM"))
        psc = ctx.enter_context(tc.tile_pool(name="psc", bufs=1, space="PSUM"))
        pgt = ctx.enter_context(tc.tile_pool(name="pgt", bufs=1, space="PSUM"))

        # resident weights / biases: ONE dma each for the packed tensors,
        # issued from the Act queue so SP can start input loads in parallel
        wall = wpool.tile([128, 6, 2, HD], f32, tag="wall")
        nc.scalar.dma_start(wall[:].bitcast(f32r),
                            wpack_d[:, :, :, :].rearrange("w i p o -> p (w i) o")
                            .bitcast(f32r))
        w_sb = {wn: [wall[:, wi, 0], wall[:, wi, 1]]
                for wi, wn in enumerate(WNAMES)}
        ball = wpool.tile([128, 4, 2], f32, tag="ball")
        nc.sync.dma_start(ball[:], bpack_d[:, :, :].rearrange("b j p -> p b j"))
        b_sb = {bn: [ball[:, bi, 0:1], ball[:, bi, 1:2]]
                for bi, bn in enumerate(BNAMES)}

        zconst = wpool.tile([128, 1], f32, tag="zconst")
        nc.vector.memset(zconst[:], 0.0)
        halfpi = wpool.tile([128, 1], f32, tag="halfpi")
        nc.vector.memset(halfpi[:], PI / 2)

        def emit_tail(pend, bias_ap):
            """Deferred atan2 tail for a previous group: arctan + quadrant
            corrections + sign, returns (av, axo_tile, dram_ap)."""
            u, Sc_all, Ss_all, pg0, pt0 = pend
            Sc, Ss = fl(Sc_all), fl(Ss_all)
            av = a2p.tile([128, GH, 2, TB], f32, tag="a2")
            if bias_ap is None:
                nc.scalar.activation(fl(av), fl(u), AF.Arctan)
            else:
                nc.scalar.activation(fl(av), fl(u), AF.Arctan, bias=bias_ap)
            th = outp.tile([128, GH, 2, TB], f32, tag="axo")
            nc.vector.tensor_scalar(fl(th), fl(av), 1.0, PI / 4, ALU.mult, ALU.add)
            e2 = a2p.tile([128, GH, 2, TB], f32, tag="a2")
            nc.vector.tensor_scalar(fl(e2), fl(av), -1.0, 0.75 * PI, ALU.mult, ALU.add)
            xn = a2i.tile([128, GH, 2, TB], i32, tag="msk")
            nc.gpsimd.tensor_scalar(fl(xn), Sc, -C_ABS, None, ALU.is_lt)
            nc.vector.copy_predicated(fl(th), fl(xn), fl(e2))
            sb = a2i.tile([128, GH, 2, TB], i32, tag="msk")
            nc.vector.tensor_scalar(fl(sb), Ss.bitcast(i32), -0x80000000, None,
                                    ALU.bitwise_and)
            nc.vector.tensor_tensor(fl(th).bitcast(i32), fl(th).bitcast(i32),
                                    fl(sb), ALU.bitwise_or)
            dram = axo_d[pg0 * HD:(pg0 + GH) * HD, pt0:pt0 + TB].rearrange(
                "(h j p) t -> p h j t", p=128, j=2)
            return av, th, dram

        tok_exp = None   # [tok_b2a0, tok_b2a1, tok_nb2g0, tok_nb2g1] or None
        pend_tail = None  # (u, Sc_all, Ss_all, g0, t0) awaiting arctan
        pend_dmas = []    # [(dram_ap, tile)] output DMAs to trigger next group
        for bt in range(NBT):
            t0 = bt * TB
            for g0 in range(0, HEADS, GH):
                heads = list(range(g0, g0 + GH))
                at_h, gt_h, ex_h = {}, {}, {}
                minv_all = bat.tile([128, GH, 2, TB], f32, tag="minv")
                eg_all = bat.tile([128, GH, 2, TB], f32, tag="eg")
                Sc_all = scp.tile([128, GH, 2, TB], f32, tag="Sc")
                Ss_all = scp.tile([128, GH, 2, TB], f32, tag="Ss")

                # ---- input DMAs for the whole group first, THEN deferred
                # output DMAs of the previous group (SP triggers in program
                # order; outputs would otherwise stall next group's loads).
                for hh, h in enumerate(heads):
                    at = atp.tile([128, N, 2, TB], f32, tag="at")
                    gt = gtp.tile([128, N, 2, TB], f32, tag="gt")
                    nc.sync.dma_start(
                        at[:].rearrange("p a b c -> p (a b) c").bitcast(f32r),
                        axisQ_d[h, :, :, t0:t0 + TB].bitcast(f32r))
                    nc.sync.dma_start(
                        gt[:].rearrange("p a b c -> p (a b) c").bitcast(f32r),
                        argQ_d[h, :, :, t0:t0 + TB].bitcast(f32r))
                    at_h[h], gt_h[h] = at, gt
                for dram, tile_ in pend_dmas:
                    nc.sync.dma_start(dram, tile_[:])
                pend_dmas = []

                # ------------- PHASE 1 (table: exp_and_others) -------------
                for hh, h in enumerate(heads):
                    at, gt = at_h[h], gt_h[h]

                    # L1 axis branch -> relu (Act)
                    h1a = []
                    for j in range(2):
                        pa = pmm.tile([128, N, TB], f32, tag="pmm")
                        for n in range(N):
                            k = 0
                            for wn in ("waax", "waar"):
                                for i in range(2):
                                    rhs = at[:, n, i, :] if wn == "waax" else gt[:, n, i, :]
                                    nc.tensor.matmul(
                                        pa[:, n, :],
                                        w_sb[wn][i][:, j * 128:(j + 1) * 128].bitcast(f32r),
                                        rhs.bitcast(f32r),
                                        start=(k == 0), stop=(k == 3))
                                    k += 1
                        ht = h1p.tile([128, N, TB], f32, tag="h1")
                        if j == 0:
                            nc.scalar.activation(ht[:].bitcast(f32r), pa[:], AF.Relu,
                                                 bias=b_sb["b1a"][j][:])
                        else:
                            nc.vector.tensor_scalar(
                                ht[:].rearrange("p a b -> p (a b)").bitcast(f32r),
                                pa[:].rearrange("p a b -> p (a b)"),
                                b_sb["b1a"][j][:], 0.0, ALU.add, ALU.max)
                        h1a.append(ht)

                    # L2 axis scores -> exp
                    ex = expp.tile([128, N, 2, TB], f32, tag="ex")
                    for j in range(2):
                        ps = psc.tile([128, N, TB], f32, tag="psc")
                        for n in range(N):
                            for i in range(2):
                                nc.tensor.matmul(
                                    ps[:, n, :],
                                    w_sb["w2a"][i][:, j * 128:(j + 1) * 128].bitcast(f32r),
                                    h1a[i][:, n, :].bitcast(f32r),
                                    start=(i == 0), stop=(i == 1))
                        bias_ap = (tok_exp[j][:] if tok_exp is not None
                                   else b_sb["b2a"][j][:])
                        nc.scalar.activation(ex[:, :, j, :], ps[:], AF.Exp,
                                             bias=bias_ap)
                    ex_h[h] = ex

                    # L1 arg branch -> relu (Pool)
                    h1g = []
                    for j in range(2):
                        pa = pmm.tile([128, N, TB], f32, tag="pmm")
                        for n in range(N):
                            k = 0
                            for wn in ("wgax", "wgar"):
                                for i in range(2):
                                    rhs = at[:, n, i, :] if wn == "wgax" else gt[:, n, i, :]
                                    nc.tensor.matmul(
                                        pa[:, n, :],
                                        w_sb[wn][i][:, j * 128:(j + 1) * 128].bitcast(f32r),
                                        rhs.bitcast(f32r),
                                        start=(k == 0), stop=(k == 3))
                                    k += 1
                        ht = h1p.tile([128, N, TB], f32, tag="h1")
                        # Pool cannot read PSUM; relu-g on DVE
                        nc.vector.tensor_scalar(
                            ht[:].rearrange("p a b -> p (a b)"),
                            pa[:].rearrange("p a b -> p (a b)"),
                            b_sb["b1g"][j][:], 0.0, ALU.add, ALU.max)
                        h1g.append(ht)

                    # arg gate: sum h1g over n (Pool), tiny L2g matmul, exp(-z)
                    hs = []
                    for i in range(2):
                        l1t = hsp.tile([128, 2, TB], f32, tag="hsl1")
                        nc.gpsimd.tensor_tensor(l1t[:], h1g[i][:, 0:2, :],
                                                h1g[i][:, 2:4, :], ALU.add)
                        st = hsp.tile([128, TB], f32, tag="hsl2")
                        nc.gpsimd.tensor_tensor(st[:].bitcast(f32r), l1t[:, 0, :],
                                                l1t[:, 1, :], ALU.add)
                        hs.append(st)
                    pg = pgt.tile([128, 2, TB], f32, tag="pgt")
                    for j in range(2):
                        for i in range(2):
                            nc.tensor.matmul(
                                pg[:, j, :],
                                w_sb["w2g"][i][:, j * 128:(j + 1) * 128].bitcast(f32r),
                                hs[i][:].bitcast(f32r),
                                start=(i == 0), stop=(i == 1))
                    for j in range(2):
                        bias_ap = (tok_exp[2 + j][:] if tok_exp is not None
                                   else b_sb["nb2g"][j][:])
                        nc.scalar.activation(eg_all[:, hh, j, :], pg[:, j, :], AF.Exp,
                                             scale=-1.0, bias=bias_ap)

                    # min over n (DVE; Pool lacks TT-min)
                    ml = l1p.tile([128, 2, 2, TB], f32, tag="lvl1")
                    nc.vector.tensor_tensor(fl(ml), fl(gt[:, 0:2]), fl(gt[:, 2:4]), ALU.min)
                    nc.vector.tensor_tensor(minv_all[:, hh].rearrange("p a b -> p (a b)"),
                                            fl(ml[:, 0:1]), fl(ml[:, 1:2]), ALU.min)


                # ---- token cascade: trig acts must follow this group's exps.
                # tokS carries 0.0 (sin bias), tokC pi/2 (cos bias). Each
                # [128,2] source slice covers both j-halves of an exp/eg pair.
                tok = None
                for hh, h in enumerate(heads):
                    for src in (ex_h[h][:, 0, :, 0], eg_all[:, hh, :, 0]):
                        nt = tokp.tile([128, 2], f32, tag="tok")
                        nc.scalar.activation(nt[:], src, AF.Identity, scale=0.0,
                                             bias=(zconst[:] if tok is None
                                                   else tok[:, 0:1]))
                        tok = nt
                tokS = tok[:, 0:1]
                tokC_t = tokp.tile([128, 1], f32, tag="tokc")
                nc.scalar.activation(tokC_t[:], tokS, AF.Identity, scale=0.0,
                                     bias=halfpi[:])
                tokC = tokC_t[:]

                # ------------- PHASE 2 (table: trig_and_small) -------------
                trig_h = {}
                for hh, h in enumerate(heads):
                    at, ex = at_h[h], ex_h[h]
                    # no range reduction: Sin table accurate to |x|~3.2 (x~N(0,1));
                    # cos(x) = Sin(pi/2 - |x|), arg in [-4.8, pi/2] stays in range.
                    last = (hh == GH - 1)
                    sinv = trp.tile([128, N, 2, TB], f32, tag="tr")
                    if last:
                        sacc = tokp.tile([128, 1], f32, tag="sacc")
                        nc.scalar.activation(fl(sinv), fl(at), AF.Sin, bias=tokS,
                                             accum_out=sacc[:])
                    else:
                        nc.scalar.activation(fl(sinv), fl(at), AF.Sin, bias=tokS)
                    axabs = wvp.tile([128, N, 2, TB], f32, tag="wv")
                    nc.vector.tensor_scalar(fl(axabs).bitcast(i32), fl(at).bitcast(i32),
                                            0x7FFFFFFF, None, ALU.bitwise_and)
                    cosv = trp.tile([128, N, 2, TB], f32, tag="tr")
                    if last:
                        cacc = tokp.tile([128, 1], f32, tag="sacc")
                        nc.scalar.activation(fl(cosv), fl(axabs), AF.Sin, scale=-1.0,
                                             bias=tokC, accum_out=cacc[:])
                        trig_acc = (sacc, cacc)
                    else:
                        nc.scalar.activation(fl(cosv), fl(axabs), AF.Sin, scale=-1.0,
                                             bias=tokC)
                        trig_acc = None
                    # in-place products: cosv <- ex*cosv (=ec), sinv <- ex*sinv (=es)
                    ec, es = cosv, sinv
                    nc.gpsimd.tensor_tensor(fl(ec), fl(ex), fl(cosv), ALU.mult)
                    nc.gpsimd.tensor_tensor(fl(es), fl(ex), fl(sinv), ALU.mult)
                    # Sc/Ss trees (Pool)
                    c1 = l1p.tile([128, 2, 2, TB], f32, tag="lvl1")
                    nc.gpsimd.tensor_tensor(fl(c1), fl(ec[:, 0:2]), fl(ec[:, 2:4]), ALU.add)
                    nc.gpsimd.tensor_tensor(Sc_all[:, hh].rearrange("p a b -> p (a b)"),
                                            fl(c1[:, 0:1]), fl(c1[:, 1:2]), ALU.add)
                    s1 = l1p.tile([128, 2, 2, TB], f32, tag="lvl1")
                    nc.gpsimd.tensor_tensor(fl(s1), fl(es[:, 0:2]), fl(es[:, 2:4]), ALU.add)
                    nc.gpsimd.tensor_tensor(Ss_all[:, hh].rearrange("p a b -> p (a b)"),
                                            fl(s1[:, 0:1]), fl(s1[:, 1:2]), ALU.add)
                    trig_h[h] = trig_acc

                # ---- deferred arctan tail of the previous group: rides the
                # TAIL of this trig-table run (chained after our last cos act
                # so no extra table load is needed for Arctan).
                av_prev = None
                if pend_tail is not None:
                    cacc_hB = trig_h[heads[-1]][1]
                    ztok = tokp.tile([128, 1], f32, tag="ztok")
                    nc.scalar.activation(ztok[:], cacc_hB[:], AF.Identity,
                                         scale=0.0, bias=zconst[:])
                    av_prev, pth, pdram = emit_tail(pend_tail, ztok[:])
                    pend_dmas.append((pdram, pth))
                    pend_tail = None

                # ---- atan2 prep (arctan itself deferred to next group):
                # atan2(|y|,|x|) = pi/4 + atan((|y|-|x|)/(|y|+|x|))
                Sc, Ss = fl(Sc_all), fl(Ss_all)
                Ac = a2p.tile([128, GH, 2, TB], f32, tag="a2")
                nc.vector.tensor_scalar(fl(Ac).bitcast(i32), Sc.bitcast(i32),
                                        0x7FFFFFFF, None, ALU.bitwise_and)
                nc.vector.tensor_scalar(fl(Ac), fl(Ac), C_ABS, None, ALU.max)
                Bt = a2p.tile([128, GH, 2, TB], f32, tag="a2")
                nc.vector.tensor_scalar(fl(Bt).bitcast(i32), Ss.bitcast(i32),
                                        0x7FFFFFFF, None, ALU.bitwise_and)
                nm = a2p.tile([128, GH, 2, TB], f32, tag="a2")
                nc.gpsimd.tensor_tensor(fl(nm), fl(Bt), fl(Ac), ALU.subtract)
                dn = a2p.tile([128, GH, 2, TB], f32, tag="a2")
                nc.gpsimd.tensor_tensor(fl(dn), fl(Bt), fl(Ac), ALU.add)
                u = up.tile([128, GH, 2, TB], f32, tag="u")
                nc.vector.reciprocal(fl(u), fl(dn))
                nc.gpsimd.tensor_tensor(fl(u), fl(nm), fl(u), ALU.mult)
                pend_tail = (u, Sc_all, Ss_all, g0, t0)

                # ---- gate finish: arg_out = min / (1 + exp(-z)); DMA deferred
                den = a2p.tile([128, GH, 2, TB], f32, tag="a2")
                nc.gpsimd.tensor_scalar(fl(den), fl(eg_all), 1.0, None, ALU.add)
                nc.vector.reciprocal(fl(den), fl(den))
                ao = outp.tile([128, GH, 2, TB], f32, tag="ago")
                nc.gpsimd.tensor_tensor(fl(ao), fl(minv_all), fl(den), ALU.mult)
                pend_dmas.append((
                    ago_d[g0 * HD:(g0 + GH) * HD, t0:t0 + TB].rearrange(
                        "(h j p) t -> p h j t", p=128, j=2), ao))

                # ---- tokens: next group's exps must follow this group's
                # sin/cos acts (via their accum side-outputs) and the
                # previous group's arctan.
                hB = heads[-1]
                srcs = [trig_h[hB][1][:], trig_h[hB][1][:],
                        (av_prev[:, 0, 0, 0:1] if av_prev is not None
                         else trig_h[hB][1][:]),
                        trig_h[hB][1][:]]
                tok_exp = []
                for (bname, j), src in zip(
                        (("b2a", 0), ("b2a", 1), ("nb2g", 0), ("nb2g", 1)), srcs):
                    nt = tokp.tile([128, 1], f32, tag="tokx")
                    nc.scalar.activation(nt[:], src, AF.Identity,
                                         scale=0.0, bias=b_sb[bname][j][:])
                    tok_exp.append(nt)

        # final flush; spread the last DMAs across engine queues
        if pend_tail is not None:
            _, pth, pdram = emit_tail(pend_tail, None)
            pend_dmas.append((pdram, pth))
        qs = [nc.sync, nc.gpsimd]
        for qi, (dram, tile_) in enumerate(pend_dmas):
            qs[qi % len(qs)].dma_start(dram, tile_[:])

    nc.compile()
    return nc


def _get_nc():
    if "nc" not in _CACHE:
        _CACHE["nc"] = _build()
    return _CACHE["nc"]


def _prep_host(axis_embeddings, arg_embeddings, W_axis1, b_axis1, W_arg1, b_arg1,
               W_axis2, b_axis2, W_arg2, b_arg2):
    f = np.float32
    W_axis1 = np.asarray(W_axis1, f); W_arg1 = np.asarray(W_arg1, f)
    W_axis2 = np.asarray(W_axis2, f); W_arg2 = np.asarray(W_arg2, f)
    # logits = [axis - arg/2, axis + arg/2]; fold concat into effective weights
    wmats = [
        (W_axis1[:, :HD] + W_axis1[:, HD:]).T,            # waax
        ((W_axis1[:, HD:] - W_axis1[:, :HD]) / 2).T,      # waar
        (W_arg1[:, :HD] + W_arg1[:, HD:]).T,              # wgax
        ((W_arg1[:, HD:] - W_arg1[:, :HD]) / 2).T,        # wgar
        W_axis2.T,                                        # w2a
        (W_arg2 / N).T,                                   # w2g (folds mean)
    ]
    wpack = np.ascontiguousarray(
        np.stack([w.reshape(2, 128, HD) for w in wmats]).astype(f))
    bpack = np.ascontiguousarray(np.stack([
        np.asarray(b_axis1, f).reshape(2, 128),
        np.asarray(b_arg1, f).reshape(2, 128),
        np.asarray(b_axis2, f).reshape(2, 128),
        -np.asarray(b_arg2, f).reshape(2, 128),
    ]))
    weights = {"wpack": wpack, "bpack": bpack}
    axis_embeddings = np.asarray(axis_embeddings, f)
    arg_embeddings = np.asarray(arg_embeddings, f)

    def to_q(x):      # [N, BL, DIM] -> [HEADS, 128, N*2, BL]
        v = x.reshape(N, BL, HEADS, 2, 128)
        return np.ascontiguousarray(v.transpose(2, 4, 0, 3, 1).reshape(
            HEADS, 128, N * 2, BL))

    in_maps = []
    for c in range(NCORES):
        sl = slice(c * BL, (c + 1) * BL)
        m = dict(weights)
        m["axisQ"] = to_q(axis_embeddings[:, sl, :])
        m["argQ"] = to_q(arg_embeddings[:, sl, :])
        in_maps.append(m)
    return in_maps


def kernel(axis_embeddings, arg_embeddings, W_axis1, b_axis1, W_arg1, b_arg1,
           W_axis2, b_axis2, W_arg2, b_arg2, _return_results=False):
    from concourse.bass_utils import run_bass_kernel_spmd
    nc = _get_nc()
    in_maps = _prep_host(axis_embeddings, arg_embeddings, W_axis1, b_axis1,
                         W_arg1, b_arg1, W_axis2, b_axis2, W_arg2, b_arg2)
    res = run_bass_kernel_spmd(nc, in_maps, list(range(NCORES)))
    f = np.float32
    axis_out = np.empty((B, DIM), f)
    arg_out = np.empty((B, DIM), f)
    for c in range(NCORES):
        sl = slice(c * BL, (c + 1) * BL)
        axis_out[sl] = res.results[c]["axis_outT"].T
        arg_out[sl] = res.results[c]["arg_outT"].T
    if _return_results:
        return (axis_out, arg_out), res
    return axis_out, arg_out


# revision 73
# speedup vs baseline: 1.0159x; 1.0159x over previous
"""Trainium2 Bass kernel for nn_ConeIntersection (v2).

Pure data-parallel over B (8 cores x 1024 tokens). Host pre-transposes inputs
to [N, DIM, B_local]; concat([axis-arg/2, axis+arg/2]) folded into effective
weights; mean-over-N of the gate folded into summed h1g + scaled W.

v2 engine plan (vs baseline: DVE 400us / Act 286us / PE 189us / Pool 0):
 - Two activation tables only (exp_and_others / trig_and_small), grouped per
   2-head phase via token-activation dependency chains (baseline thrashed 65
   table loads = 83us).
 - atan2 via half-angle identity atan2(|y|,|x|) = pi/4 + atan((|y|-|x|)/(|y|+|x|)),
   constant clamp (drops the Se softmax-denominator entirely), sign applied
   by bit-or; DVE `divide` (2x mode) instead of reciprocal+mult.
 - sin/cos range reduction via 2-op tensor_scalar (add,mod) at 2x instead of
   add_range_wrap microcode; sin(x)=Sin(w-pi), cos(x)=Sin(w2-pi).
 - Pool engine (idle in baseline) takes arg-relu, e*cos/e*sin products,
   Ss/min/h1g trees and atan2 tensor-tensor ops.
 - Gate sigmoid via exp table + divide: min/(1+exp(-z)).
"""
import sys
sys.path.insert(0, '/opt/trn_rl_repo')
import numpy as np
from contextlib import ExitStack

N, B, DIM, HEADS = 4, 8192, 1024, 4
HD = DIM // HEADS            # 256
NCORES = 8
BL = B // NCORES             # 1024 tokens per core
TB = 256                     # token tile (matmul free dim; >=256 keeps f32r at 1cyc/row)
NBT = BL // TB               # 4
GH = 2                       # heads per activation-table phase group
PI = 3.141592653589793
C_ABS = 1e-6                 # abs clamp for x_emb (replaces 0.001*sum_exp; see notes)
USE_MOD_WRAP = False

_CACHE = {}


def _build():
    from concourse import bacc, tile, mybir
    f32 = mybir.dt.float32
    f32r = mybir.dt.float32r
    i32 = mybir.dt.int32
    AF = mybir.ActivationFunctionType
    ALU = mybir.AluOpType

    nc = bacc.Bacc("TRN2", target_bir_lowering=False, debug=False,
                   num_devices=NCORES)

    # host pre-lays inputs as [head, partition, (n,j), token] so one 3-dim
    # DMA loads a full head tile [128, N, 2, TB]
    axisQ_d = nc.dram_tensor("axisQ", [HEADS, 128, N * 2, BL], f32,
                             kind="ExternalInput")
    argQ_d = nc.dram_tensor("argQ", [HEADS, 128, N * 2, BL], f32,
                            kind="ExternalInput")
    # all weights packed into one dram tensor: [6 W, 2 i-chunks, 128, HD],
    # all biases into another: [4, 2, 128]
    wpack_d = nc.dram_tensor("wpack", [6, 2, 128, HD], f32, kind="ExternalInput")
    bpack_d = nc.dram_tensor("bpack", [4, 2, 128], f32, kind="ExternalInput")
    WNAMES = ["waax", "waar", "wgax", "wgar", "w2a", "w2g"]
    BNAMES = ["b1a", "b1g", "b2a", "nb2g"]
    axo_d = nc.dram_tensor("axis_outT", [DIM, BL], f32, kind="ExternalOutput")
    ago_d = nc.dram_tensor("arg_outT", [DIM, BL], f32, kind="ExternalOutput")

    fl = lambda t: t[:].rearrange("p a b c -> p (a b c)")

    with tile.TileContext(nc) as tc, ExitStack() as ctx:
        wpool = ctx.enter_context(tc.tile_pool(name="w", bufs=1))
        atp = ctx.enter_context(tc.tile_pool(name="atp", bufs=3))     # 8KB each
        gtp = ctx.enter_context(tc.tile_pool(name="gtp", bufs=2))     # 8KB
        h1p = ctx.enter_context(tc.tile_pool(name="h1p", bufs=3))     # 4KB
        expp = ctx.enter_context(tc.tile_pool(name="expp", bufs=2))   # 8KB
        wvp = ctx.enter_context(tc.tile_pool(name="wvp", bufs=1))     # 8KB wrap args
        trp = ctx.enter_context(tc.tile_pool(name="trp", bufs=4))     # 8KB trig transients
        l1p = ctx.enter_context(tc.tile_pool(name="l1p", bufs=2))     # 4KB tree lvl1
        hsp = ctx.enter_context(tc.tile_pool(name="hsp", bufs=2))     # h1g sums
        scp = ctx.enter_context(tc.tile_pool(name="scp", bufs=2))     # 4KB Sc/Ss group tiles
        bat = ctx.enter_context(tc.tile_pool(name="bat", bufs=1))     # 4KB minv/eg group
        a2p = ctx.enter_context(tc.tile_pool(name="a2p", bufs=4))     # 4KB atan2 temps
        a2i = ctx.enter_context(tc.tile_pool(name="a2i", bufs=2))     # 4KB i32 masks
        up = ctx.enter_context(tc.tile_pool(name="up", bufs=2))       # 4KB deferred u
        outp = ctx.enter_context(tc.tile_pool(name="outp", bufs=2))   # 4KB outputs
        tokp = ctx.enter_context(tc.tile_pool(name="tokp", bufs=4))   # tiny
        pmm = ctx.enter_context(tc.tile_pool(name="pmm", bufs=2, space="PSUM"))
        psc = ctx.enter_context(tc.tile_pool(name="psc", bufs=1, space="PSUM"))
        pgt = ctx.enter_context(tc.tile_pool(name="pgt", bufs=1, space="PSUM"))

        # resident weights / biases: ONE dma each for the packed tensors,
        # issued from the Act queue so SP can start input loads in parallel
        wall = wpool.tile([128, 6, 2, HD], f32, tag="wall")
        nc.scalar.dma_start(wall[:].bitcast(f32r),
                            wpack_d[:, :, :, :].rearrange("w i p o -> p (w i) o")
                            .bitcast(f32r))
        w_sb = {wn: [wall[:, wi, 0], wall[:, wi, 1]]
                for wi, wn in enumerate(WNAMES)}
        ball = wpool.tile([128, 4, 2], f32, tag="ball")
        nc.sync.dma_start(ball[:], bpack_d[:, :, :].rearrange("b j p -> p b j"))
        b_sb = {bn: [ball[:, bi, 0:1], ball[:, bi, 1:2]]
                for bi, bn in enumerate(BNAMES)}

        zconst = wpool.tile([128, 1], f32, tag="zconst")
        nc.vector.memset(zconst[:], 0.0)
        halfpi = wpool.tile([128, 1], f32, tag="halfpi")
        nc.vector.memset(halfpi[:], PI / 2)

        def emit_tail(pend, bias_ap):
            """Deferred atan2 tail for a previous group: arctan + quadrant
            corrections + sign, returns (av, axo_tile, dram_ap)."""
            u, Sc_all, Ss_all, pg0, pt0 = pend
            Sc, Ss = fl(Sc_all), fl(Ss_all)
            av = a2p.tile([128, GH, 2, TB], f32, tag="a2")
            if bias_ap is None:
                nc.scalar.activation(fl(av), fl(u), AF.Arctan)
            else:
                nc.scalar.activation(fl(av), fl(u), AF.Arctan, bias=bias_ap)
            th = outp.tile([128, GH, 2, TB], f32, tag="axo")
            nc.vector.tensor_scalar(fl(th), fl(av), 1.0, PI / 4, ALU.mult, ALU.add)
            e2 = a2p.tile([128, GH, 2, TB], f32, tag="a2")
            nc.vector.tensor_scalar(fl(e2), fl(av), -1.0, 0.75 * PI, ALU.mult, ALU.add)
            xn = a2i.tile([128, GH, 2, TB], i32, tag="msk")
            nc.gpsimd.tensor_scalar(fl(xn), Sc, -C_ABS, None, ALU.is_lt)
            nc.vector.copy_predicated(fl(th), fl(xn), fl(e2))
            sb = a2i.tile([128, GH, 2, TB], i32, tag="msk")
            nc.vector.tensor_scalar(fl(sb), Ss.bitcast(i32), -0x80000000, None,
                                    ALU.bitwise_and)
            nc.vector.tensor_tensor(fl(th).bitcast(i32), fl(th).bitcast(i32),
                                    fl(sb), ALU.bitwise_or)
            dram = axo_d[pg0 * HD:(pg0 + GH) * HD, pt0:pt0 + TB].rearrange(
                "(h j p) t -> p h j t", p=128, j=2)
            return av, th, dram

        tok_exp = None   # [tok_b2a0, tok_b2a1, tok_nb2g0, tok_nb2g1] or None
        pend_tail = None  # (u, Sc_all, Ss_all, g0, t0) awaiting arctan
        pend_dmas = []    # [(dram_ap, tile)] output DMAs to trigger next group
        for bt in range(NBT):
            t0 = bt * TB
            for g0 in range(0, HEADS, GH):
                heads = list(range(g0, g0 + GH))
                at_h, gt_h, ex_h = {}, {}, {}
                minv_all = bat.tile([128, GH, 2, TB], f32, tag="minv")
                eg_all = bat.tile([128, GH, 2, TB], f32, tag="eg")
                Sc_all = scp.tile([128, GH, 2, TB], f32, tag="Sc")
                Ss_all = scp.tile([128, GH, 2, TB], f32, tag="Ss")

                # ---- input DMAs for the whole group first, THEN deferred
                # output DMAs of the previous group (SP triggers in program
                # order; outputs would otherwise stall next group's loads).
                for hh, h in enumerate(heads):
                    at = atp.tile([128, N, 2, TB], f32, tag="at")
                    gt = gtp.tile([128, N, 2, TB], f32, tag="gt")
                    nc.sync.dma_start(
                        at[:].rearrange("p a b c -> p (a b) c").bitcast(f32r),
                        axisQ_d[h, :, :, t0:t0 + TB].bitcast(f32r))
                    nc.sync.dma_start(
                        gt[:].rearrange("p a b c -> p (a b) c").bitcast(f32r),
                        argQ_d[h, :, :, t0:t0 + TB].bitcast(f32r))
                    at_h[h], gt_h[h] = at, gt
                for dram, tile_ in pend_dmas:
                    nc.sync.dma_start(dram, tile_[:])
                pend_dmas = []

                # ------------- PHASE 1 (table: exp_and_others) -------------
                for hh, h in enumerate(heads):
                    at, gt = at_h[h], gt_h[h]

                    # L1 axis branch -> relu (Act)
                    h1a = []
                    for j in range(2):
                        pa = pmm.tile([128, N, TB], f32, tag="pmm")
                        for n in range(N):
                            k = 0
                            for wn in ("waax", "waar"):
                                for i in range(2):
                                    rhs = at[:, n, i, :] if wn == "waax" else gt[:, n, i, :]
                                    nc.tensor.matmul(
                                        pa[:, n, :],
                                        w_sb[wn][i][:, j * 128:(j + 1) * 128].bitcast(f32r),
                                        rhs.bitcast(f32r),
                                        start=(k == 0), stop=(k == 3))
                                    k += 1
                        ht = h1p.tile([128, N, TB], f32, tag="h1")
                        if j == 0:
                            nc.scalar.activation(ht[:].bitcast(f32r), pa[:], AF.Relu,
                                                 bias=b_sb["b1a"][j][:])
                        else:
                            nc.vector.tensor_scalar(
                                ht[:].rearrange("p a b -> p (a b)").bitcast(f32r),
                                pa[:].rearrange("p a b -> p (a b)"),
                                b_sb["b1a"][j][:], 0.0, ALU.add, ALU.max)
                        h1a.append(ht)

                    # L2 axis scores -> exp
                    ex = expp.tile([128, N, 2, TB], f32, tag="ex")
                    for j in range(2):
                        ps = psc.tile([128, N, TB], f32, tag="psc")
                        for n in range(N):
                            for i in range(2):
                                nc.tensor.matmul(
                                    ps[:, n, :],
                                    w_sb["w2a"][i][:, j * 128:(j + 1) * 128].bitcast(f32r),
                                    h1a[i][:, n, :].bitcast(f32r),
                                    start=(i == 0), stop=(i == 1))
                        bias_ap = (tok_exp[j][:] if tok_exp is not None
                                   else b_sb["b2a"][j][:])
                        nc.scalar.activation(ex[:, :, j, :], ps[:], AF.Exp,
                                             bias=bias_ap)
                    ex_h[h] = ex

                    # L1 arg branch -> relu (Pool)
                    h1g = []
                    for j in range(2):
                        pa = pmm.tile([128, N, TB], f32, tag="pmm")
                        for n in range(N):
                            k = 0
                            for wn in ("wgax", "wgar"):
                                for i in range(2):
                                    rhs = at[:, n, i, :] if wn == "wgax" else gt[:, n, i, :]
                                    nc.tensor.matmul(
                                        pa[:, n, :],
                                        w_sb[wn][i][:, j * 128:(j + 1) * 128].bitcast(f32r),
                                        rhs.bitcast(f32r),
                                        start=(k == 0), stop=(k == 3))
                                    k += 1
                        ht = h1p.tile([128, N, TB], f32, tag="h1")
                        # Pool cannot read PSUM; relu-g on DVE
                        nc.vector.tensor_scalar(
                            ht[:].rearrange("p a b -> p (a b)"),
                            pa[:].rearrange("p a b -> p (a b)"),
                            b_sb["b1g"][j][:], 0.0, ALU.add, ALU.max)
                        h1g.append(ht)

                    # arg gate: sum h1g over n (Pool), tiny L2g matmul, exp(-z)
                    hs = []
                    for i in range(2):
                        l1t = hsp.tile([128, 2, TB], f32, tag="hsl1")
                        nc.gpsimd.tensor_tensor(l1t[:], h1g[i][:, 0:2, :],
                                                h1g[i][:, 2:4, :], ALU.add)
                        st = hsp.tile([128, TB], f32, tag="hsl2")
                        nc.gpsimd.tensor_tensor(st[:].bitcast(f32r), l1t[:, 0, :],
                                                l1t[:, 1, :], ALU.add)
                        hs.append(st)
                    pg = pgt.tile([128, 2, TB], f32, tag="pgt")
                    for j in range(2):
                        for i in range(2):
                            nc.tensor.matmul(
                                pg[:, j, :],
                                w_sb["w2g"][i][:, j * 128:(j + 1) * 128].bitcast(f32r),
                                hs[i][:].bitcast(f32r),
                                start=(i == 0), stop=(i == 1))
                    for j in range(2):
                        bias_ap = (tok_exp[2 + j][:] if tok_exp is not None
                                   else b_sb["nb2g"][j][:])
                        nc.scalar.activation(eg_all[:, hh, j, :], pg[:, j, :], AF.Exp,
                                             scale=-1.0, bias=bias_ap)

                    # min over n (DVE; Pool lacks TT-min)
                    ml = l1p.tile([128, 2, 2, TB], f32, tag="lvl1")
                    nc.vector.tensor_tensor(fl(ml), fl(gt[:, 0:2]), fl(gt[:, 2:4]), ALU.min)
                    nc.vector.tensor_tensor(minv_all[:, hh].rearrange("p a b -> p (a b)"),
                                            fl(ml[:, 0:1]), fl(ml[:, 1:2]), ALU.min)


                # ---- token cascade: trig acts must follow this group's exps.
                # tokS carries 0.0 (sin bias), tokC pi/2 (cos bias). Each
                # [128,2] source slice covers both j-halves of an exp/eg pair.
                tok = None
                for hh, h in enumerate(heads):
                    for src in (ex_h[h][:, 0, :, 0], eg_all[:, hh, :, 0]):
                        nt = tokp.tile([128, 2], f32, tag="tok")
                        nc.scalar.activation(nt[:], src, AF.Identity, scale=0.0,
                                             bias=(zconst[:] if tok is None
                                                   else tok[:, 0:1]))
                        tok = nt
                tokS = tok[:, 0:1]
                tokC_t = tokp.tile([128, 1], f32, tag="tokc")
                nc.scalar.activation(tokC_t[:], tokS, AF.Identity, scale=0.0,
                                     bias=halfpi[:])
                tokC = tokC_t[:]

                # ------------- PHASE 2 (table: trig_and_small) -------------
                trig_h = {}
                for hh, h in enumerate(heads):
                    at, ex = at_h[h], ex_h[h]
                    # no range reduction: Sin table accurate to |x|~3.2 (x~N(0,1));
                    # cos(x) = Sin(pi/2 - |x|), arg in [-4.8, pi/2] stays in range.
                    last = (hh == GH - 1)
                    sinv = trp.tile([128, N, 2, TB], f32, tag="tr")
                    if last:
                        sacc = tokp.tile([128, 1], f32, tag="sacc")
                        nc.scalar.activation(fl(sinv), fl(at), AF.Sin, bias=tokS,
                                             accum_out=sacc[:])
                    else:
                        nc.scalar.activation(fl(sinv), fl(at), AF.Sin, bias=tokS)
                    axabs = wvp.tile([128, N, 2, TB], f32, tag="wv")
                    nc.vector.tensor_scalar(fl(axabs).bitcast(i32), fl(at).bitcast(i32),
                                            0x7FFFFFFF, None, ALU.bitwise_and)
                    cosv = trp.tile([128, N, 2, TB], f32, tag="tr")
                    if last:
                        cacc = tokp.tile([128, 1], f32, tag="sacc")
                        nc.scalar.activation(fl(cosv), fl(axabs), AF.Sin, scale=-1.0,
                                             bias=tokC, accum_out=cacc[:])
                        trig_acc = (sacc, cacc)
                    else:
                        nc.scalar.activation(fl(cosv), fl(axabs), AF.Sin, scale=-1.0,
                                             bias=tokC)
                        trig_acc = None
                    # in-place products: cosv <- ex*cosv (=ec), sinv <- ex*sinv (=es)
                    ec, es = cosv, sinv
                    nc.gpsimd.tensor_tensor(fl(ec), fl(ex), fl(cosv), ALU.mult)
                    nc.gpsimd.tensor_tensor(fl(es), fl(ex), fl(sinv), ALU.mult)
                    # Sc/Ss trees (Pool)
                    c1 = l1p.tile([128, 2, 2, TB], f32, tag="lvl1")
                    nc.gpsimd.tensor_tensor(fl(c1), fl(ec[:, 0:2]), fl(ec[:, 2:4]), ALU.add)
                    nc.gpsimd.tensor_tensor(Sc_all[:, hh].rearrange("p a b -> p (a b)"),
                                            fl(c1[:, 0:1]), fl(c1[:, 1:2]), ALU.add)
                    s1 = l1p.tile([128, 2, 2, TB], f32, tag="lvl1")
                    nc.gpsimd.tensor_tensor(fl(s1), fl(es[:, 0:2]), fl(es[:, 2:4]), ALU.add)
                    nc.gpsimd.tensor_tensor(Ss_all[:, hh].rearrange("p a b -> p (a b)"),
                                            fl(s1[:, 0:1]), fl(s1[:, 1:2]), ALU.add)
                    trig_h[h] = trig_acc

                # ---- deferred arctan tail of the previous group: rides the
                # TAIL of this trig-table run (chained after our last cos act
                # so no extra table load is needed for Arctan).
                av_prev = None
                if pend_tail is not None:
                    cacc_hB = trig_h[heads[-1]][1]
                    ztok = tokp.tile([128, 1], f32, tag="ztok")
                    nc.scalar.activation(ztok[:], cacc_hB[:], AF.Identity,
                                         scale=0.0, bias=zconst[:])
                    av_prev, pth, pdram = emit_tail(pend_tail, ztok[:])
                    pend_dmas.append((pdram, pth))
                    pend_tail = None

                # ---- atan2 prep (arctan itself deferred to next group):
                # atan2(|y|,|x|) = pi/4 + atan((|y|-|x|)/(|y|+|x|))
                Sc, Ss = fl(Sc_all), fl(Ss_all)
                Ac = a2p.tile([128, GH, 2, TB], f32, tag="a2")
                nc.vector.tensor_scalar(fl(Ac).bitcast(i32), Sc.bitcast(i32),
                                        0x7FFFFFFF, None, ALU.bitwise_and)
                nc.vector.tensor_scalar(fl(Ac), fl(Ac), C_ABS, None, ALU.max)
                Bt = a2p.tile([128, GH, 2, TB], f32, tag="a2")
                nc.vector.tensor_scalar(fl(Bt).bitcast(i32), Ss.bitcast(i32),
                                        0x7FFFFFFF, None, ALU.bitwise_and)
                nm = a2p.tile([128, GH, 2, TB], f32, tag="a2")
                nc.gpsimd.tensor_tensor(fl(nm), fl(Bt), fl(Ac), ALU.subtract)
                dn = a2p.tile([128, GH, 2, TB], f32, tag="a2")
                nc.gpsimd.tensor_tensor(fl(dn), fl(Bt), fl(Ac), ALU.add)
                u = up.tile([128, GH, 2, TB], f32, tag="u")
                nc.vector.reciprocal(fl(u), fl(dn))
                nc.gpsimd.tensor_tensor(fl(u), fl(nm), fl(u), ALU.mult)
                pend_tail = (u, Sc_all, Ss_all, g0, t0)

                # ---- gate finish: arg_out = min / (1 + exp(-z)); DMA deferred
                den = a2p.tile([128, GH, 2, TB], f32, tag="a2")
                nc.gpsimd.tensor_scalar(fl(den), fl(eg_all), 1.0, None, ALU.add)
                nc.vector.reciprocal(fl(den), fl(den))
                ao = outp.tile([128, GH, 2, TB], f32, tag="ago")
                nc.gpsimd.tensor_tensor(fl(ao), fl(minv_all), fl(den), ALU.mult)
                pend_dmas.append((
                    ago_d[g0 * HD:(g0 + GH) * HD, t0:t0 + TB].rearrange(
                        "(h j p) t -> p h j t", p=128, j=2), ao))

                # ---- tokens: next group's exps must follow this group's
                # sin/cos acts (via their accum side-outputs) and the
                # previous group's arctan.
                hB = heads[-1]
                srcs = [trig_h[hB][1][:], trig_h[hB][1][:],
                        (av_prev[:, 0, 0, 0:1] if av_prev is not None
                         else trig_h[hB][1][:]),
                        trig_h[hB][1][:]]
                tok_exp = []
                for (bname, j), src in zip(
                        (("b2a", 0), ("b2a", 1), ("nb2g", 0), ("nb2g", 1)), srcs):
                    nt = tokp.tile([128, 1], f32, tag="tokx")
                    nc.scalar.activation(nt[:], src, AF.Identity,
                                         scale=0.0, bias=b_sb[bname][j][:])
                    tok_exp.append(nt)

        # final flush; spread the last DMAs across engine queues
        if pend_tail is not None:
            _, pth, pdram = emit_tail(pend_tail, None)
            pend_dmas.append((pdram, pth))
        qs = [nc.sync, nc.gpsimd]
        for qi, (dram, tile_) in enumerate(pend_dmas):
            qs[qi % len(qs)].dma_start(dram, tile_[:])

    nc.compile()
    return nc


def _get_nc():
    if "nc" not in _CACHE:
        _CACHE["nc"] = _build()
    return _CACHE["nc"]


def _prep_host(axis_embeddings, arg_embeddings, W_axis1, b_axis1, W_arg1, b_arg1,
               W_axis2, b_axis2, W_arg2, b_arg2):
    f = np.float32
    W_axis1 = np.asarray(W_axis1, f); W_arg1 = np.asarray(W_arg1, f)
    W_axis2 = np.asarray(W_axis2, f); W_arg2 = np.asarray(W_arg2, f)
    # logits = [axis - arg/2, axis + arg/2]; fold concat into effective weights
    wmats = [
        (W_axis1[:, :HD] + W_axis1[:, HD:]).T,            # waax
        ((W_axis1[:, HD:] - W_axis1[:, :HD]) / 2).T,      # waar
        (W_arg1[:, :HD] + W_arg1[:, HD:]).T,              # wgax
        ((W_arg1[:, HD:] - W_arg1[:, :HD]) / 2).T,        # wgar
        W_axis2.T,                                        # w2a
        (W_arg2 / N).T,                                   # w2g (folds mean)
    ]
    wpack = np.ascontiguousarray(
        np.stack([w.reshape(2, 128, HD) for w in wmats]).astype(f))
    bpack = np.ascontiguousarray(np.stack([
        np.asarray(b_axis1, f).reshape(2, 128),
        np.asarray(b_arg1, f).reshape(2, 128),
        np.asarray(b_axis2, f).reshape(2, 128),
        -np.asarray(b_arg2, f).reshape(2, 128),
    ]))
    weights = {"wpack": wpack, "bpack": bpack}
    axis_embeddings = np.asarray(axis_embeddings, f)
    arg_embeddings = np.asarray(arg_embeddings, f)

    def to_q(x):      # [N, BL, DIM] -> [HEADS, 128, N*2, BL]
        v = x.reshape(N, BL, HEADS, 2, 128)
        return np.ascontiguousarray(v.transpose(2, 4, 0, 3, 1).reshape(
            HEADS, 128, N * 2, BL))

    in_maps = []
    for c in range(NCORES):
        sl = slice(c * BL, (c + 1) * BL)
        m = dict(weights)
        m["axisQ"] = to_q(axis_embeddings[:, sl, :])
        m["argQ"] = to_q(arg_embeddings[:, sl, :])
        in_maps.append(m)
    return in_maps


def kernel(axis_embeddings, arg_embeddings, W_axis1, b_axis1, W_arg1, b_arg1,
           W_axis2, b_axis2, W_arg2, b_arg2, _return_results=False):
    from concourse.bass_utils import run_bass_kernel_spmd
    nc = _get_nc()
    in_maps = _prep_host(axis_embeddings, arg_embeddings, W_axis1, b_axis1,
                         W_arg1, b_arg1, W_axis2, b_axis2, W_arg2, b_arg2)
    res = run_bass_kernel_spmd(nc, in_maps, list(range(NCORES)))
    f = np.float32
    axis_out = np.empty((B, DIM), f)
    arg_out = np.empty((B, DIM), f)
    for c in range(NCORES):
        sl = slice(c * BL, (c + 1) * BL)
        axis_out[sl] = res.results[c]["axis_outT"].T
        arg_out[sl] = res.results[c]["arg_outT"].T
    if _return_results:
        return (axis_out, arg_out), res
    return axis_out, arg_out
